# revision 1
# baseline (speedup 1.0000x reference)
"""Multi-head attention kernel for Trainium2, SPMD across 8 NeuronCores.

Problem: b=2, s=2048, d_model=1024, 16 heads x 64 dims, packed QKV proj,
softmax over keys (boolean key mask), out-projection.

Sharding: core c in 0..7 handles batch b = c//4 and a group of 4 heads
g = c%4 (data parallel over batch x head/tensor parallel).  Each core
computes its head-group's out-projection partial [2048, 1024]; the host
sums the 4 partials per batch (the row-parallel reduction) and upcasts
from bf16.

Device-side dataflow per core (bf16 matmul operands, fp32 PSUM):
  - QKV proj, weights stationary.  Q,K produced transposed [d, s], two
    heads packed per SBUF tile (head A rows 0-63, head B rows 64-127).
    V produced in natural layout [s, d] as 16 tiles [128, 4*65] with a
    ones-column per head (col 64) that makes the PV matmul also produce
    the softmax rowsum.  Masked key rows of V (and the ones col) are
    zeroed via a per-partition scalar multiply == exact -inf masking.
  - Scores transposed St[sk, sq]: per (pair, sq-half, key-tile) two
    [128, 1024] PSUM tiles (one per head); the two heads run as
    row-tiled concurrent matmuls (tile_position rows 0/64).  Double
    buffered so exp streams back-to-back on ScalarE.
  - exp on ScalarE (scale=1/8), FD=1024 per instr, PSUM -> bf16 SBUF.
  - PV: out^T[65, sq] accumulated over key tiles in PSUM; row 64 =
    rowsum.  PV trails St/exp by PIPE iterations so PSUM-slot waits at
    loop boundaries never head-of-line-block the PE FIFO.
  - normalize: rowsum row -> SBUF (DVE), DMA-hop to partition 0
    (GPSIMD partition_broadcast's ucode can only read partitions 0-15),
    partition_broadcast to 64 rows, reciprocal_approx_fast (DVE custom
    op; must NOT read PSUM on HW), multiply -> O^T packed per pair
    (head B staged through SBUF scratch + DMA into rows 64-127).
  - out-proj: stationary = packed O^T s-slices [128, 128], moving =
    W_out^T, both pairs accumulated in PSUM; evict split across DVE and
    ScalarE; bf16 DMA to DRAM.

Schedule notes: V projection is injected into the first attention
j-loop (PV trails by 8 there) so exp starts right after the Q/K
projections; input DMAs are emitted in first-use order and chunked so
the first matmul waits only on its own slice.  PSUM budget: 2x
[128,1024] score tiles + 2x [65,1024] accumulators = 8 banks.
"""

import numpy as np
import ml_dtypes

BF = ml_dtypes.bfloat16
S = 2048
C = 1024
DQ = 64
HL = 4  # local heads per core
KT = S // 128  # 16 key tiles
CT = C // 128  # 8 contraction tiles
SCALE = 8.0  # sqrt(DQ)

_CACHED = None


def _build():
    import concourse.bacc as bacc
    import concourse.mybir as mybir
    import concourse.tile as tile

    F32 = mybir.dt.float32
    BF16 = mybir.dt.bfloat16
    EXP = mybir.ActivationFunctionType.Exp

    nc = bacc.Bacc(
        "TRN2",
        target_bir_lowering=False,
        debug=False,
        enable_asserts=False,
        num_devices=8,
    )

    XT = nc.dram_tensor("xt", [C, S], BF16, kind="ExternalInput").ap()
    WQ = nc.dram_tensor("wq", [128, CT * 256], BF16, kind="ExternalInput").ap()
    WK = nc.dram_tensor("wk", [128, CT * 256], BF16, kind="ExternalInput").ap()
    WV = nc.dram_tensor("wv", [C, 2 * 128], BF16, kind="ExternalInput").ap()
    WO = nc.dram_tensor("wo", [HL * DQ, C], BF16, kind="ExternalInput").ap()
    MV = nc.dram_tensor("maskv", [128, KT], F32, kind="ExternalInput").ap()
    OUT = nc.dram_tensor("out", [S, C], BF16, kind="ExternalOutput").ap()

    with tile.TileContext(nc) as tc:
        with (
            tc.tile_pool(name="xt", bufs=CT) as p_xt,
            tc.tile_pool(name="wqk", bufs=2) as p_w,
            tc.tile_pool(name="wv", bufs=CT) as p_wv,
            tc.tile_pool(name="wo", bufs=2) as p_wo,
            tc.tile_pool(name="cst", bufs=1) as p_c,
            tc.tile_pool(name="qk", bufs=4) as p_qk,
            tc.tile_pool(name="v", bufs=KT) as p_v,
            tc.tile_pool(name="pt", bufs=18) as p_pt,
            tc.tile_pool(name="r", bufs=1) as p_r,
            tc.tile_pool(name="bc", bufs=1) as p_bc,
            tc.tile_pool(name="ot", bufs=2) as p_ot,
            tc.tile_pool(name="sc", bufs=1) as p_sc,
            tc.tile_pool(name="os", bufs=4) as p_os,
            tc.tile_pool(name="psA", bufs=2, space="PSUM") as psA,
            tc.tile_pool(name="psB", bufs=2, space="PSUM") as psB,
        ):
            # ---------------- input DMA ----------------
            # Order matters: the first projection matmul waits on wq + xt[0],
            # so emit those DMAs first; wv/wo/mask are needed much later.
            wq_sb = p_w.tile([128, CT * 256], BF16, tag="wq", name="wq_sb")
            wk_sb = p_w.tile([128, CT * 256], BF16, tag="wk", name="wk_sb")
            xt_t = [p_xt.tile([128, S], BF16, tag="xt", name="xt_t") for _ in range(CT)]
            # chunked so the first projection matmuls wait only on their slice
            for c in range(CT):
                nc.sync.dma_start(
                    wq_sb[:, c * 256 : (c + 1) * 256], WQ[:, c * 256 : (c + 1) * 256]
                )
                nc.sync.dma_start(
                    xt_t[c][:, 0:1024], XT[c * 128 : (c + 1) * 128, 0:1024]
                )
                nc.sync.dma_start(
                    xt_t[c][:, 1024:2048], XT[c * 128 : (c + 1) * 128, 1024:2048]
                )
                nc.sync.dma_start(
                    wk_sb[:, c * 256 : (c + 1) * 256], WK[:, c * 256 : (c + 1) * 256]
                )
            wv_t = []
            for c in range(CT):
                t = p_wv.tile([128, HL * DQ], BF16, tag="wv", name="wv_t")
                nc.sync.dma_start(t[:], WV[c * 128 : (c + 1) * 128, :])
                wv_t.append(t)
            mv_t = p_c.tile([128, KT], F32, tag="mv", name="mv_t")
            nc.sync.dma_start(mv_t[:], MV[:])
            wo_t = []
            for p in range(2):
                t = p_wo.tile([128, C], BF16, tag="wo", name="wo_t")
                nc.sync.dma_start(t[:], WO[p * 128 : (p + 1) * 128, :])
                wo_t.append(t)

            # ---------------- QKV projection ----------------
            qk_tiles = {}

            def proj_qk(nm, wsb, pair, pools):
                dst = p_qk.tile([128, S], BF16, tag="qk", name="qk_t")
                qk_tiles[(nm, pair)] = dst
                ps_tiles = [
                    (pools[0].tile([128, 1024], F32, tag=pools[1], name="pp"), 0),
                    (pools[0].tile([128, 1024], F32, tag=pools[1], name="pp"), 1024),
                ]
                for c in range(CT):
                    wt = wsb[:, c * 256 + pair * 128 : c * 256 + (pair + 1) * 128]
                    for pst, off in ps_tiles:
                        for n in range(2):
                            nc.tensor.matmul(
                                pst[:, n * 512 : (n + 1) * 512],
                                lhsT=wt,
                                rhs=xt_t[c][:, off + n * 512 : off + (n + 1) * 512],
                                start=(c == 0),
                                stop=(c == CT - 1),
                            )
                for pst, off in ps_tiles:
                    nc.vector.tensor_copy(dst[:, off : off + 1024], pst[:, 0:1024])

            def proj_v_tile(st):
                psv = psB.tile([128, HL * DQ], F32, tag="B", name="psv")
                for c in range(CT):
                    nc.tensor.matmul(
                        psv[:, 0 : HL * DQ],
                        lhsT=xt_t[c][:, st * 128 : (st + 1) * 128],
                        rhs=wv_t[c][:],
                        start=(c == 0),
                        stop=(c == CT - 1),
                    )
                vt = p_v.tile([128, HL * 65], BF16, tag="v", name="v_t")
                v3 = vt[:, 0 : HL * 65].rearrange("p (h c) -> p h c", c=65)
                s3 = psv[:, 0 : HL * DQ].rearrange("p (h c) -> p h c", c=DQ)
                nc.vector.tensor_copy(v3[:, :, 0:DQ], s3[:, :, :])
                nc.vector.memset(v3[:, :, DQ : DQ + 1], 1.0)
                nc.vector.tensor_scalar_mul(vt[:], vt[:], mv_t[:, st : st + 1])
                v_t.append(vt)

            v_t = []
            proj_qk("q", wq_sb, 0, (psA, "A"))
            proj_qk("k", wk_sb, 0, (psB, "B"))
            proj_qk("q", wq_sb, 1, (psA, "A"))
            proj_qk("k", wk_sb, 1, (psB, "B"))
            # V projection is emitted INSIDE the first attention j-loop
            # (see attention()), so the exp stream starts right after the
            # Q/K projections and V projects under the first 8 exps.

            # ---------------- attention ----------------
            # Per (pair, j-half): ping-pong St tiles [128,1024] per head so
            # exp (ScalarE) streams back-to-back while the PE computes the
            # next scores; PV trails PIPE iterations behind so a blocked acc
            # slot at a j-boundary doesn't head-of-line-block St in the PE
            # FIFO.
            PIPE = 2
            rth = [p_r.tile([65, S], F32, tag="rA", name="r_t"),
                   p_r.tile([65, S], F32, tag="rB", name="r_t")]
            bct_i = [p_bc.tile([64, 1024], F32, tag="bcA", name="bc_t"),
                     p_bc.tile([64, 1024], F32, tag="bcB", name="bc_t")]
            bc2_i = [p_bc.tile([64, 1024], F32, tag="bc2A", name="bc2_t"),
                     p_bc.tile([64, 1024], F32, tag="bc2B", name="bc2_t")]
            ot_tiles = []
            scr = p_sc.tile([64, S], BF16, tag="sc", name="sc_t")

            def attention(pair):
                qt = qk_tiles[("q", pair)]
                kt = qk_tiles[("k", pair)]
                ot = p_ot.tile([128, S], BF16, tag="ot", name="ot_t")
                ot_tiles.append(ot)
                hA, hB = 2 * pair, 2 * pair + 1
                for j in range(2):
                    # On the very first j-loop, inject the V and pair-1 Q/K
                    # projections into the PE stream after the first 8
                    # score tiles, so exp runs while they project.  PV then
                    # trails by 8 (it needs V).
                    inject = pair == 0 and j == 0
                    pipe = 8 if inject else PIPE
                    jo = j * 1024
                    accs = []
                    pts = {}

                    def st_exp(k):
                        for i, base in enumerate((0, 64)):
                            stp = psA.tile([128, 1024], F32, tag="A", name="stp")
                            for n in range(2):
                                nc.tensor.matmul(
                                    stp[:, n * 512 : (n + 1) * 512],
                                    lhsT=kt[base : base + DQ, k * 128 : (k + 1) * 128],
                                    rhs=qt[base : base + DQ, jo + n * 512 : jo + (n + 1) * 512],
                                    start=True,
                                    stop=True,
                                )
                            pt = p_pt.tile([128, 1024], BF16, tag="pt", name="pt_t")
                            nc.scalar.activation(pt[:], stp[:], EXP, scale=1.0 / SCALE)
                            pts[(k, i)] = pt

                    def pv(k):
                        if not accs:
                            accs.append(psB.tile([65, 1024], F32, tag="B", name="acc"))
                            accs.append(psB.tile([65, 1024], F32, tag="B", name="acc"))
                        for i, h in enumerate((hA, hB)):
                            pt = pts.pop((k, i))
                            for n in range(2):
                                nc.tensor.matmul(
                                    accs[i][0:65, n * 512 : (n + 1) * 512],
                                    lhsT=v_t[k][:, h * 65 : h * 65 + 65],
                                    rhs=pt[:, n * 512 : (n + 1) * 512],
                                    start=(k == 0),
                                    stop=(k == KT - 1),
                                )

                    for k in range(KT):
                        st_exp(k)
                        if inject and k == 7:
                            for st in range(KT):
                                proj_v_tile(st)
                        if k >= pipe:
                            pv(k - pipe)
                    for k in range(KT - pipe, KT):
                        pv(k)

                    # normalize:  O = PV / rowsum  (rowsum in acc row 64).
                    # partition_broadcast's ucode reads via gpsimd core 0,
                    # which only sees physical partitions 0-15 -> the source
                    # row must sit on partition 0; DMA-hop it there first.
                    dsts = (ot[0:64, :], scr[0:64, :])
                    for i in range(2):
                        acc, dst = accs[i], dsts[i]
                        bct, bc2 = bct_i[i], bc2_i[i]
                        nc.vector.tensor_copy(
                            rth[i][64:65, jo : jo + 1024], acc[64:65, 0:1024]
                        )
                        nc.sync.dma_start(
                            rth[i][0:1, jo : jo + 1024], rth[i][64:65, jo : jo + 1024]
                        )
                        nc.gpsimd.partition_broadcast(
                            bct[0:64, 0:1024], rth[i][0:1, jo : jo + 1024]
                        )
                        nc.vector.reciprocal_approx_fast(
                            bc2[0:64, 0:1024], bct[0:64, 0:1024]
                        )
                        nc.vector.tensor_mul(
                            dst[:, jo : jo + 1024], acc[0:64, 0:1024], bc2[0:64, 0:1024]
                        )
                    # pack head B into rows 64..127 of the pair's O tile
                    nc.sync.dma_start(
                        ot[64:128, jo : jo + 1024], scr[0:64, jo : jo + 1024]
                    )

            attention(0)
            attention(1)

            # ---------------- out-projection ----------------
            ps_cycle = [(psA, "A"), (psB, "B")]
            for st in range(KT):
                pool, tag = ps_cycle[st % 2]
                po = pool.tile([128, C], F32, tag=tag, name="po")
                for p in range(2):
                    for n in range(2):
                        nc.tensor.matmul(
                            po[:, n * 512 : (n + 1) * 512],
                            lhsT=ot_tiles[p][:, st * 128 : (st + 1) * 128],
                            rhs=wo_t[p][:, n * 512 : (n + 1) * 512],
                            start=(p == 0),
                            stop=(p == 1),
                        )
                os_t = p_os.tile([128, C], BF16, tag="os", name="os_t")
                nc.vector.tensor_copy(os_t[:, 0:512], po[:, 0:512])
                nc.scalar.copy(os_t[:, 512:1024], po[:, 512:1024])
                nc.sync.dma_start(OUT[st * 128 : (st + 1) * 128, :], os_t[:])

    nc.compile()
    return nc


def _get_nc():
    global _CACHED
    if _CACHED is None:
        _CACHED = _build()
    return _CACHED


def _prep_in_maps(X, W_qkv, W_out, mask):
    X = np.asarray(X, dtype=np.float32)
    Wqkv = np.asarray(W_qkv, dtype=np.float32)
    Wo = np.asarray(W_out, dtype=np.float32)
    m = np.asarray(mask)
    W3 = Wqkv.reshape(16, DQ, 3, C)
    in_maps = []
    for core in range(8):
        b = core // 4
        g = core % 4
        hs = slice(4 * g, 4 * g + 4)
        wq = W3[hs, :, 0, :].reshape(HL * DQ, C).T.astype(BF)
        wk = W3[hs, :, 1, :].reshape(HL * DQ, C).T.astype(BF)
        # pre-arrange for contiguous SBUF prestage: [128, c*256+j]
        wq = np.ascontiguousarray(
            wq.reshape(CT, 128, HL * DQ).transpose(1, 0, 2).reshape(128, CT * 256)
        )
        wk = np.ascontiguousarray(
            wk.reshape(CT, 128, HL * DQ).transpose(1, 0, 2).reshape(128, CT * 256)
        )
        wv = np.ascontiguousarray(W3[hs, :, 2, :].reshape(HL * DQ, C).T.astype(BF))
        wo = np.ascontiguousarray(Wo[:, 256 * g : 256 * (g + 1)].T.astype(BF))
        xt = np.ascontiguousarray(X[b].T.astype(BF))
        mv = np.ascontiguousarray(
            m[b].astype(np.float32).reshape(KT, 128).T
        )
        in_maps.append(
            {"xt": xt, "wq": wq, "wk": wk, "wv": wv, "wo": wo, "maskv": mv}
        )
    return in_maps


def _run(in_maps, trace=False, **kw):
    from concourse import bass_utils

    nc = _get_nc()
    return bass_utils.run_bass_kernel_spmd(
        nc, in_maps, core_ids=list(range(8)), trace=trace, **kw
    )


def _gather(results):
    out = np.empty((2, S, C), dtype=np.float32)
    p = [r["out"].astype(np.float32) for r in results]
    out[0] = p[0] + p[1] + p[2] + p[3]
    out[1] = p[4] + p[5] + p[6] + p[7]
    return out


def kernel(X, W_qkv, W_out, mask):
    in_maps = _prep_in_maps(X, W_qkv, W_out, mask)
    res = _run(in_maps)
    return _gather(res.results)



# revision 2
# speedup vs baseline: 1.1075x; 1.1075x over previous
"""Multi-head attention kernel for Trainium2, SPMD across 8 NeuronCores.

Problem: b=2, s=2048, d_model=1024, 16 heads x 64 dims, packed QKV proj,
softmax over keys (boolean key mask), out-projection.

Sharding: core c in 0..7 handles batch b = c//4 and a group of 4 heads
g = c%4 (data parallel over batch x head/tensor parallel).  Each core
computes its head-group's out-projection partial [2048, 1024]; the host
sums the 4 partials per batch (the row-parallel reduction) and upcasts
from bf16.

Key scheduling facts (from NTFF traces of the previous version):
  - The PE streams ~2 cols/ns (throttle-capped ~83% of 2.4GHz); total
    matmul column count (~390k) sets a ~200us floor.
  - The attention inner loop was SCALAR-bound: 2 exps of [128,1024] per
    key tile = 2.56us vs 2.08us of PE work.  Fusing both heads' scores
    into one [128,2048] PSUM tile -> ONE ACT instr (2.19us) rebalances.
  - DMA triggers cost ~0.7us each on the issuing queue; input loads are
    split across the two HWDGE queues (sync + scalar), outputs are
    batched 4 row-tiles per DMA.
  - The tail (normalize chain + all 16 out-proj tiles + 16 output DMAs
    after the last PV) was ~28us; out-proj tiles 0-11 are now injected
    into the last attention loops, which are sq-split 512-wide so their
    PSUM fits St+accs+po simultaneously.

Device-side dataflow per core (bf16 matmul operands, fp32 PSUM):
  - QKV proj, weights stationary.  Q,K produced transposed [d, s], two
    heads packed per SBUF tile (head A rows 0-63, head B rows 64-127).
    V produced in natural layout [s, d] as 16 tiles [128, 4*65] with a
    ones-column per head (col 64) that makes the PV matmul also produce
    the softmax rowsum.  Masked key rows of V (and the ones col) are
    zeroed via a per-partition scalar multiply == exact -inf masking.
  - Attention loops j-major: (p0,j0 +V inject, pipe=16), (p1,j0),
    (p0,j1), then pair1-j1 split into two 512-col chunk loops carrying
    the out-proj injections.  Scores transposed St[sk, sq] per head into
    one shared PSUM tile; one fused exp (ScalarE, scale=1/8) per k.
  - PV: out^T[65, sq] accumulated per (head, 512-chunk) in 1-bank PSUM
    accs; row 64 = rowsum.
  - normalize per 512-chunk: rowsum row -> SBUF (DVE), DMA-hop to
    partition 0, gpsimd partition_broadcast, reciprocal_approx_fast,
    multiply -> O^T packed per pair (head B staged via scr + DMA into
    rows 64-127).
  - out-proj per s-row-tile: po [128,512] psum, stationary = packed O^T
    s-slices, moving = W_out^T, both pairs accumulated; evict split
    across ScalarE/DVE into [128, 4096] group buffers; one output DMA
    per 4 tiles (3D access pattern).
"""

import numpy as np
import ml_dtypes

BF = ml_dtypes.bfloat16
S = 2048
C = 1024
DQ = 64
HL = 4  # local heads per core
KT = S // 128  # 16 key tiles
CT = C // 128  # 8 contraction tiles
SCALE = 8.0  # sqrt(DQ)

_CACHED = None


def _build():
    import concourse.bacc as bacc
    import concourse.mybir as mybir
    import concourse.tile as tile

    F32 = mybir.dt.float32
    BF16 = mybir.dt.bfloat16
    EXP = mybir.ActivationFunctionType.Exp

    nc = bacc.Bacc(
        "TRN2",
        target_bir_lowering=False,
        debug=False,
        enable_asserts=False,
        num_devices=8,
    )

    XT = nc.dram_tensor("xt", [C, S], BF16, kind="ExternalInput").ap()
    WQ = nc.dram_tensor("wq", [128, CT * 256], BF16, kind="ExternalInput").ap()
    WK = nc.dram_tensor("wk", [128, CT * 256], BF16, kind="ExternalInput").ap()
    WV = nc.dram_tensor("wv", [128, CT * 256], BF16, kind="ExternalInput").ap()
    WO = nc.dram_tensor("wo", [HL * DQ, C], BF16, kind="ExternalInput").ap()
    MV = nc.dram_tensor("maskv", [128, KT], F32, kind="ExternalInput").ap()
    OUT = nc.dram_tensor("out", [S, C], BF16, kind="ExternalOutput").ap()

    with tile.TileContext(nc) as tc:
        with (
            tc.tile_pool(name="xt", bufs=CT) as p_xt,
            tc.tile_pool(name="w", bufs=3) as p_w,
            tc.tile_pool(name="wo", bufs=2) as p_wo,
            tc.tile_pool(name="cst", bufs=1) as p_c,
            tc.tile_pool(name="qk", bufs=4) as p_qk,
            tc.tile_pool(name="v", bufs=KT) as p_v,
            tc.tile_pool(name="pt", bufs=16) as p_pt,
            tc.tile_pool(name="r", bufs=4) as p_r,
            tc.tile_pool(name="bc", bufs=4) as p_bc,
            tc.tile_pool(name="ot", bufs=2) as p_ot,
            tc.tile_pool(name="sc", bufs=2) as p_sc,
            tc.tile_pool(name="os", bufs=2) as p_os,
            tc.tile_pool(name="psA", bufs=1, space="PSUM") as psA,
            tc.tile_pool(name="psB", bufs=4, space="PSUM") as psB,
        ):
            # ---------------- input DMA ----------------
            # Two HWDGE queues: sync gets the q-projection critical path
            # (wq + xt), scalar gets everything needed later (wk, wv,
            # mask, wo).  Triggers cost ~0.7us each on the issuing queue.
            wq_sb = p_w.tile([128, CT * 256], BF16, tag="w", name="wq_sb")
            wk_sb = p_w.tile([128, CT * 256], BF16, tag="w", name="wk_sb")
            wv_sb = p_w.tile([128, CT * 256], BF16, tag="w", name="wv_sb")
            xt_t = [p_xt.tile([128, S], BF16, tag="xt", name="xt_t") for _ in range(CT)]
            nc.sync.dma_start(wq_sb[:, 0:512], WQ[:, 0:512])
            nc.sync.dma_start(xt_t[0][:, 0:1024], XT[0:128, 0:1024])
            nc.sync.dma_start(xt_t[0][:, 1024:2048], XT[0:128, 1024:2048])
            nc.sync.dma_start(wq_sb[:, 512 : CT * 256], WQ[:, 512 : CT * 256])
            for c in range(1, CT):
                nc.sync.dma_start(xt_t[c][:], XT[c * 128 : (c + 1) * 128, :])
            nc.scalar.dma_start(wk_sb[:, 0:512], WK[:, 0:512])
            nc.scalar.dma_start(wk_sb[:, 512 : CT * 256], WK[:, 512 : CT * 256])
            nc.scalar.dma_start(wv_sb[:], WV[:])
            mv_t = p_c.tile([128, KT], F32, tag="mv", name="mv_t")
            nc.scalar.dma_start(mv_t[:], MV[:])
            wo_t = []
            for p in range(2):
                t = p_wo.tile([128, C], BF16, tag="wo", name="wo_t")
                nc.scalar.dma_start(t[:], WO[p * 128 : (p + 1) * 128, :])
                wo_t.append(t)

            # ---------------- QKV projection ----------------
            # Alternate PSUM pools per call: psA holds one [128,2048]
            # (tag A, 4 banks), psB-calls use 4x [128,512] (tag B, 1 bank
            # each).  Evictions split ScalarE/DVE so the next call's
            # PSUM frees fast.
            qk_tiles = {}

            def evict(dst, src, idx):
                if idx % 2 == 0:
                    nc.vector.tensor_copy(dst, src)
                else:
                    nc.scalar.copy(dst, src)

            def proj_qk_A(nm, wsb, pair):
                dst = p_qk.tile([128, S], BF16, tag="qk", name="qk_t")
                qk_tiles[(nm, pair)] = dst
                ps = psA.tile([128, 2048], F32, tag="A", name="pjA")
                for c in range(CT):
                    wt = wsb[:, c * 256 + pair * 128 : c * 256 + (pair + 1) * 128]
                    for q in range(4):
                        nc.tensor.matmul(
                            ps[:, q * 512 : (q + 1) * 512],
                            lhsT=wt,
                            rhs=xt_t[c][:, q * 512 : (q + 1) * 512],
                            start=(c == 0),
                            stop=(c == CT - 1),
                        )
                for q in range(4):
                    evict(dst[:, q * 512 : (q + 1) * 512], ps[:, q * 512 : (q + 1) * 512], q)

            def proj_qk_B(nm, wsb, pair):
                dst = p_qk.tile([128, S], BF16, tag="qk", name="qk_t")
                qk_tiles[(nm, pair)] = dst
                ps = [psB.tile([128, 512], F32, tag="B", name="pjB") for _ in range(4)]
                for c in range(CT):
                    wt = wsb[:, c * 256 + pair * 128 : c * 256 + (pair + 1) * 128]
                    for q in range(4):
                        nc.tensor.matmul(
                            ps[q][:, 0:512],
                            lhsT=wt,
                            rhs=xt_t[c][:, q * 512 : (q + 1) * 512],
                            start=(c == 0),
                            stop=(c == CT - 1),
                        )
                for q in range(4):
                    evict(dst[:, q * 512 : (q + 1) * 512], ps[q][:, 0:512], q)

            proj_qk_A("q", wq_sb, 0)
            proj_qk_B("k", wk_sb, 0)
            proj_qk_A("q", wq_sb, 1)
            proj_qk_B("k", wk_sb, 1)

            # ---------------- V projection (injected into loop 0) ------
            v_t = []

            def proj_v_tile(st):
                psv = psB.tile([128, HL * DQ], F32, tag="B", name="psv")
                for c in range(CT):
                    nc.tensor.matmul(
                        psv[:, 0 : HL * DQ],
                        lhsT=xt_t[c][:, st * 128 : (st + 1) * 128],
                        rhs=wv_sb[:, c * 256 : c * 256 + HL * DQ],
                        start=(c == 0),
                        stop=(c == CT - 1),
                    )
                vt = p_v.tile([128, HL * 65], BF16, tag="v", name="v_t")
                v3 = vt[:, 0 : HL * 65].rearrange("p (h c) -> p h c", c=65)
                s3 = psv[:, 0 : HL * DQ].rearrange("p (h c) -> p h c", c=DQ)
                nc.vector.tensor_copy(v3[:, :, 0:DQ], s3[:, :, :])
                nc.vector.memset(v3[:, :, DQ : DQ + 1], 1.0)
                nc.vector.tensor_scalar_mul(vt[:], vt[:], mv_t[:, st : st + 1])
                v_t.append(vt)

            # ---------------- out-projection tile ----------------------
            ot_tiles = {}
            os_groups = {}
            OUT3 = OUT.rearrange("(t p) c -> p t c", p=128)

            def emit_st_tile(st):
                g = st // 4
                if g not in os_groups:
                    os_groups[g] = p_os.tile([128, 4096], BF16, tag="os", name="os_g")
                osb = os_groups[g]
                lo = (st % 4) * 1024
                for n in range(2):
                    po = psB.tile([128, 512], F32, tag="B", name="po")
                    for p in range(2):
                        nc.tensor.matmul(
                            po[:, 0:512],
                            lhsT=ot_tiles[p][:, st * 128 : (st + 1) * 128],
                            rhs=wo_t[p][:, n * 512 : (n + 1) * 512],
                            start=(p == 0),
                            stop=(p == 1),
                        )
                    evict(osb[:, lo + n * 512 : lo + (n + 1) * 512], po[:, 0:512], n)
                if st % 4 == 3:
                    os3 = osb.rearrange("p (t c) -> p t c", c=1024)
                    nc.sync.dma_start(OUT3[:, 4 * g : 4 * g + 4, :], os3[:, :, :])

            # ---------------- attention loop ----------------------------
            # W = sq columns this loop covers (1024 fused or 512 chunk).
            # St for both heads goes into ONE psA tile [128, 2W] so exp is
            # a single ACT instruction.  PV accumulates per (head, 512-
            # chunk) into 1-bank psB accs; row 64 = rowsum.
            def attn_loop(pair, jo, W, pipe, inject=None):
                qt = qk_tiles[("q", pair)]
                kt = qk_tiles[("k", pair)]
                ot = ot_tiles[pair]
                scr = p_sc.tile([64, 1024], BF16, tag="sc", name="sc_t")
                nch = W // 512
                accs = {}
                pts = {}

                def st_exp(k):
                    stb = psA.tile([128, 2 * W], F32, tag="A", name="stb")
                    for i, base in enumerate((0, 64)):
                        for n in range(nch):
                            nc.tensor.matmul(
                                stb[:, i * W + n * 512 : i * W + (n + 1) * 512],
                                lhsT=kt[base : base + DQ, k * 128 : (k + 1) * 128],
                                rhs=qt[base : base + DQ, jo + n * 512 : jo + (n + 1) * 512],
                                start=True,
                                stop=True,
                            )
                    pt = p_pt.tile([128, 2 * W], BF16, tag="pt", name="pt_t")
                    nc.scalar.activation(pt[:], stb[:], EXP, scale=1.0 / SCALE)
                    pts[k] = pt

                def pv(k):
                    if not accs:
                        for i in range(2):
                            for n in range(nch):
                                accs[(i, n)] = psB.tile(
                                    [65, 512], F32, tag="B", name="acc"
                                )
                    pt = pts.pop(k)
                    for i, h in enumerate((2 * pair, 2 * pair + 1)):
                        for n in range(nch):
                            nc.tensor.matmul(
                                accs[(i, n)][0:65, 0:512],
                                lhsT=v_t[k][:, h * 65 : h * 65 + 65],
                                rhs=pt[:, i * W + n * 512 : i * W + (n + 1) * 512],
                                start=(k == 0),
                                stop=(k == KT - 1),
                            )

                for k in range(KT):
                    st_exp(k)
                    if inject is not None:
                        inject(k)
                    if k >= pipe:
                        pv(k - pipe)
                for k in range(max(0, KT - pipe), KT):
                    pv(k)

                # normalize per 512-chunk: rowsum (acc row 64) -> SBUF ->
                # DMA-hop to partition 0 (gpsimd ucode reads p0 only) ->
                # broadcast -> reciprocal -> multiply.
                for n in range(nch):
                    cols = jo + n * 512
                    for i in range(2):
                        acc = accs[(i, n)]
                        rth = p_r.tile([65, 512], F32, tag="r", name="r_t")
                        bct = p_bc.tile([64, 512], F32, tag="bc", name="bc_t")
                        bc2 = p_bc.tile([64, 512], F32, tag="bc", name="bc2_t")
                        nc.vector.tensor_copy(rth[64:65, 0:512], acc[64:65, 0:512])
                        nc.sync.dma_start(rth[0:1, 0:512], rth[64:65, 0:512])
                        nc.gpsimd.partition_broadcast(bct[0:64, 0:512], rth[0:1, 0:512])
                        nc.vector.reciprocal_approx_fast(bc2[0:64, 0:512], bct[0:64, 0:512])
                        dst = (
                            ot[0:64, cols : cols + 512]
                            if i == 0
                            else scr[0:64, n * 512 : (n + 1) * 512]
                        )
                        nc.vector.tensor_mul(dst, acc[0:64, 0:512], bc2[0:64, 0:512])
                    nc.sync.dma_start(
                        ot[64:128, cols : cols + 512], scr[0:64, n * 512 : (n + 1) * 512]
                    )
                accs.clear()

            for pair in range(2):
                ot_tiles[pair] = p_ot.tile([128, S], BF16, tag="ot", name="ot_t")

            # L0: pair0 j0 with V projection injected (pipe=16: all PV
            # after all V so psv/acc pool slots never deadlock).
            attn_loop(0, 0, 1024, 16, inject=lambda k: proj_v_tile(k))
            # L1: pair1 j0.
            attn_loop(1, 0, 1024, 4)
            # L2: pair0 j1.
            attn_loop(0, 1024, 1024, 4)

            # L3: pair1 j1 as two 512 chunks carrying out-proj injections.
            st_a = [0, 1, 2, 3]

            def inj_a(k):
                if k in (4, 7, 10, 13):
                    emit_st_tile(st_a.pop(0))

            attn_loop(1, 1024, 512, 4, inject=inj_a)

            st_b = [4, 5, 6, 7, 8, 9, 10, 11]

            def inj_b(k):
                if k in (2, 4, 6, 8, 10, 12, 14, 15):
                    emit_st_tile(st_b.pop(0))

            attn_loop(1, 1536, 512, 4, inject=inj_b)

            # tail: last 4 out-proj tiles after pair1-j1's final chunk.
            for st in range(12, 16):
                emit_st_tile(st)

    nc.compile()
    return nc


def _get_nc():
    global _CACHED
    if _CACHED is None:
        _CACHED = _build()
    return _CACHED


def _prep_in_maps(X, W_qkv, W_out, mask):
    X = np.asarray(X, dtype=np.float32)
    Wqkv = np.asarray(W_qkv, dtype=np.float32)
    Wo = np.asarray(W_out, dtype=np.float32)
    m = np.asarray(mask)
    W3 = Wqkv.reshape(16, DQ, 3, C)
    in_maps = []
    for core in range(8):
        b = core // 4
        g = core % 4
        hs = slice(4 * g, 4 * g + 4)
        # pre-arrange for contiguous SBUF prestage: [128, c*256 + j]
        def prearrange(w):  # w: [HL*DQ, C] -> [128, CT*256]
            wt = w.T.astype(BF)  # [C, HL*DQ]
            return np.ascontiguousarray(
                wt.reshape(CT, 128, HL * DQ).transpose(1, 0, 2).reshape(128, CT * 256)
            )

        wq = prearrange(W3[hs, :, 0, :].reshape(HL * DQ, C))
        wk = prearrange(W3[hs, :, 1, :].reshape(HL * DQ, C))
        wv = prearrange(W3[hs, :, 2, :].reshape(HL * DQ, C))
        wo = np.ascontiguousarray(Wo[:, 256 * g : 256 * (g + 1)].T.astype(BF))
        xt = np.ascontiguousarray(X[b].T.astype(BF))
        mv = np.ascontiguousarray(m[b].astype(np.float32).reshape(KT, 128).T)
        in_maps.append(
            {"xt": xt, "wq": wq, "wk": wk, "wv": wv, "wo": wo, "maskv": mv}
        )
    return in_maps


def _run(in_maps, trace=False, **kw):
    from concourse import bass_utils

    nc = _get_nc()
    return bass_utils.run_bass_kernel_spmd(
        nc, in_maps, core_ids=list(range(8)), trace=trace, **kw
    )


def _gather(results):
    out = np.empty((2, S, C), dtype=np.float32)
    p = [r["out"].astype(np.float32) for r in results]
    out[0] = p[0] + p[1] + p[2] + p[3]
    out[1] = p[4] + p[5] + p[6] + p[7]
    return out


def kernel(X, W_qkv, W_out, mask):
    in_maps = _prep_in_maps(X, W_qkv, W_out, mask)
    res = _run(in_maps)
    return _gather(res.results)
